# revision 2
# baseline (speedup 1.0000x reference)
"""Normalized Walsh-Hadamard transform over the last dim of x: (16384, 4096) fp32.

Strategy: shard rows across 8 NeuronCores (2048 rows each). Per core, use
the Kronecker factorization H4096 = H32_a (x) H128_mc with feature index
f = 128*a + mc (a = high 5 bits, mc = low 7 bits). Both factors are applied
by TensorE, using the data-as-lhsT trick so each matmul simultaneously
TRANSPOSES (moves a 128-chunk of the free dim onto partitions) and
TRANSFORMS (contracts the old partition index against a Hadamard factor):

  DMA-in   V[(rl,a), (r4,mc)] = x[4*r4+rl, 128*a+mc]   SWDGE cast fp32->bf16
           (contiguous 512B chunks in HBM - 4x the baseline's 128B)
  stage 1  ps1[mc, (j,rl,a')] = matmul(lhsT=V-chunk, rhs=I4 (x) H32/8)
  drain 1  A = ps1 -> SBUF bf16                        ScalarE copy
  stage 2  ps2[(rl,a'), (j,m'c')] = matmul(lhsT=A-chunk, rhs=H128/8)
  drain 2  Y[(rl,a'), (r4,m'c')] = ps2                 DVE copy fp32
  DMA-out  y[4*r4+rl, 128*a'+m'c'] = Y                 HWDGE fp32, 512B chunks

No DVE transposes, no butterflies: the only non-TensorE compute is the two
PSUM drains. All Hadamard entries are +-1/8, exact in bf16; end-to-end
error is bf16 rounding of the data (~2e-3 rel), far inside the 2e-2 gate.
"""
import sys

if "/opt/trn_rl_repo" not in sys.path:
    sys.path.insert(0, "/opt/trn_rl_repo")

import numpy as np

N_CORES = 8
NF = 4096
ROWS_TOTAL = 16384
ROWS_PER_CORE = ROWS_TOTAL // N_CORES


def _hadamard(n):
    h = np.array([[1.0]], dtype=np.float64)
    while h.shape[0] < n:
        h = np.block([[h, h], [h, -h]])
    return h


def make_consts():
    import ml_dtypes
    # stage-1 rhs: contracts partition p=(rl,a) -> (rl,a'), block-diag I4 x H32
    bd = (np.kron(np.eye(4), _hadamard(32)) / 8.0).astype(ml_dtypes.bfloat16)
    # stage-2 rhs: contracts partition mc -> m'c', full H128
    hb = (_hadamard(128) / 8.0).astype(ml_dtypes.bfloat16)
    return bd, hb


def build_kernel(rows_per_core=ROWS_PER_CORE, mega_rows=128, r4_chunk=8,
                 reps=1, v_bufs=3, a_bufs=3, y_bufs=2, ps_bufs=2,
                 out_dge="sync", mode="full"):
    import concourse.tile as tile
    from concourse import bacc, mybir

    assert rows_per_core % mega_rows == 0
    n_mega = rows_per_core // mega_rows
    R4 = mega_rows // 4                  # r4 values per mega-tile
    assert R4 % r4_chunk == 0
    n_chunk = R4 // r4_chunk             # PSUM chunks per mega-tile
    FC = r4_chunk * 128                  # free width per PSUM chunk
    FM = R4 * 128                        # free width per mega-tile

    nc = bacc.Bacc("TRN2", target_bir_lowering=False, debug=False)
    x_d = nc.dram_tensor("x", [rows_per_core, NF], mybir.dt.float32,
                         kind="ExternalInput")
    bd_d = nc.dram_tensor("bd", [128, 128], mybir.dt.bfloat16,
                          kind="ExternalInput")
    hb_d = nc.dram_tensor("hb", [128, 128], mybir.dt.bfloat16,
                          kind="ExternalInput")
    y_d = nc.dram_tensor("y", [rows_per_core, NF], mybir.dt.float32,
                         kind="ExternalOutput")

    with tile.TileContext(nc) as tc:
        with (
            tc.tile_pool(name="consts", bufs=1) as cpool,
            tc.tile_pool(name="vin", bufs=v_bufs) as vpool,
            tc.tile_pool(name="amid", bufs=a_bufs) as apool,
            tc.tile_pool(name="yout", bufs=y_bufs) as ypool,
            tc.tile_pool(name="ps1", bufs=ps_bufs, space="PSUM") as ps1pool,
            tc.tile_pool(name="ps2", bufs=ps_bufs, space="PSUM") as ps2pool,
        ):
            bd_sb = cpool.tile([128, 128], mybir.dt.bfloat16)
            nc.sync.dma_start(bd_sb[:], bd_d.ap())
            hb_sb = cpool.tile([128, 128], mybir.dt.bfloat16)
            nc.sync.dma_start(hb_sb[:], hb_d.ap())

            def body(_it=None):
                for t in range(n_mega):
                    r0 = t * mega_rows
                    xi = x_d.ap()[r0:r0 + mega_rows, :].rearrange(
                        "(r4 rl) (a mc) -> (rl a) r4 mc",
                        rl=4, a=32, r4=R4, mc=128)
                    v = vpool.tile([128, FM], mybir.dt.bfloat16, tag="v")
                    nc.gpsimd.dma_start(
                        v[:].rearrange("p (r4 mc) -> p r4 mc",
                                       r4=R4, mc=128), xi)

                    y_sb = ypool.tile([128, FM], mybir.dt.float32, tag="y")
                    if mode == "dma":
                        nc.vector.memset(y_sb[:], 0.0)
                    for q in range(n_chunk if mode != "dma" else 0):
                        ps1 = ps1pool.tile([128, FC], mybir.dt.float32,
                                           tag="ps1")
                        for j in range(r4_chunk):
                            g = q * r4_chunk + j
                            nc.tensor.matmul(
                                ps1[:, j * 128:(j + 1) * 128],
                                v[:, g * 128:(g + 1) * 128], bd_sb[:])
                        if mode == "t1":
                            nc.scalar.copy(y_sb[:, q * FC:(q + 1) * FC],
                                           ps1[:])
                            continue
                        a_sb = apool.tile([128, FC], mybir.dt.bfloat16,
                                          tag="a")
                        nc.scalar.copy(a_sb[:], ps1[:])

                        ps2 = ps2pool.tile([128, FC], mybir.dt.float32,
                                           tag="ps2")
                        for j in range(r4_chunk):
                            nc.tensor.matmul(
                                ps2[:, j * 128:(j + 1) * 128],
                                a_sb[:, j * 128:(j + 1) * 128], hb_sb[:])
                        nc.vector.tensor_copy(
                            y_sb[:, q * FC:(q + 1) * FC], ps2[:])

                    yo = y_d.ap()[r0:r0 + mega_rows, :].rearrange(
                        "(r4 rl) (a mc) -> (rl a) r4 mc",
                        rl=4, a=32, r4=R4, mc=128)
                    ysrc = y_sb[:].rearrange(
                        "p (r4 mc) -> p r4 mc", r4=R4, mc=128)
                    if out_dge == "sync":
                        nc.sync.dma_start(yo, ysrc)
                    else:
                        nc.gpsimd.dma_start(yo, ysrc)

            if reps == 1:
                body()
            else:
                with tc.For_i(0, reps, 1) as it:
                    body(it)

    nc.compile()
    return nc


def kernel(x):
    from concourse.bass_utils import run_bass_kernel_spmd

    x = np.asarray(x, dtype=np.float32)
    assert x.shape == (ROWS_TOTAL, NF)
    nc = build_kernel()
    bd, hb = make_consts()
    shards = x.reshape(N_CORES, ROWS_PER_CORE, NF)
    in_maps = [
        {"x": np.ascontiguousarray(shards[i]), "bd": bd, "hb": hb}
        for i in range(N_CORES)
    ]
    res = run_bass_kernel_spmd(nc, in_maps, core_ids=list(range(N_CORES)))
    y = np.concatenate([res.results[i]["y"] for i in range(N_CORES)], axis=0)
    return np.asarray(y, dtype=np.float32)


# revision 9
# speedup vs baseline: 1.5338x; 1.5338x over previous
"""Normalized Walsh-Hadamard transform over the last dim of x: (16384, 4096) fp32.

Strategy: shard rows across 8 NeuronCores (2048 rows each). Per core, use
the Kronecker factorization H4096 = H32_a (x) H128_mc with feature index
f = 128*a + mc (a = high 5 bits, mc = low 7 bits). Both factors are applied
by TensorE, using the data-as-lhsT trick so each matmul simultaneously
TRANSPOSES (moves a 128-chunk of the free dim onto partitions) and
TRANSFORMS (contracts the old partition index against a Hadamard factor):

  DMA-in   V[(rl,a), (r4,mc)] = x[4*r4+rl, 128*a+mc]   SWDGE cast fp32->bf16
           (contiguous 512B chunks in HBM - 4x the baseline's 128B)
  stage 1  ps1[mc, (j,rl,a')] = matmul(lhsT=V-chunk, rhs=I4 (x) H32/8)
  drain 1  A = ps1 -> SBUF bf16                        ScalarE copy
  stage 2  ps2[(rl,a'), (j,m'c')] = matmul(lhsT=A-chunk, rhs=H128/8)
  drain 2  Y[(rl,a'), (r4,m'c')] = ps2                 DVE copy fp32
  DMA-out  y[4*r4+rl, 128*a'+m'c'] = Y                 HWDGE fp32, 512B chunks

No DVE transposes, no butterflies: the only non-TensorE compute is the two
PSUM drains. All Hadamard entries are +-1/8, exact in bf16; end-to-end
error is bf16 rounding of the data (~2e-3 rel), far inside the 2e-2 gate.
"""
import sys

if "/opt/trn_rl_repo" not in sys.path:
    sys.path.insert(0, "/opt/trn_rl_repo")

import numpy as np

N_CORES = 8
NF = 4096
ROWS_TOTAL = 16384
ROWS_PER_CORE = ROWS_TOTAL // N_CORES


def _hadamard(n):
    h = np.array([[1.0]], dtype=np.float64)
    while h.shape[0] < n:
        h = np.block([[h, h], [h, -h]])
    return h


def make_consts(contig_out=True):
    import ml_dtypes
    # stage-1 rhs: contracts partition p=(rl,a) -> (rl,a'), block-diag I4 x H32
    bd = np.kron(np.eye(4), _hadamard(32)) / 8.0
    if contig_out:
        # permute columns so stage-1 output lands as (j, a', rl') -- makes
        # drain1 a straight copy into A[(r4, a', rl')] and lets stage 2 put
        # ROWS on the output partitions (contiguous row-major DMA-out)
        perm = [32 * (n & 3) + (n >> 2) for n in range(128)]
        bd = bd[:, perm]
    # stage-2 rhs: contracts partition mc -> m'c', full H128
    hb = _hadamard(128) / 8.0
    return bd.astype(ml_dtypes.bfloat16), hb.astype(ml_dtypes.bfloat16)


def build_kernel(rows_per_core=ROWS_PER_CORE, mega_rows=128, r4_chunk=8,
                 reps=1, v_bufs=3, a_bufs=3, y_bufs=2, ps_bufs=2,
                 out_dge="sync", mode="full", contig_out=True):
    import concourse.tile as tile
    from concourse import bacc, mybir

    assert rows_per_core % mega_rows == 0
    n_mega = rows_per_core // mega_rows
    R4 = mega_rows // 4                  # r4 values per mega-tile
    assert R4 % r4_chunk == 0
    n_chunk = R4 // r4_chunk             # PSUM chunks per mega-tile
    FC = r4_chunk * 128                  # free width per PSUM chunk
    FM = R4 * 128                        # free width per mega-tile

    nc = bacc.Bacc("TRN2", target_bir_lowering=False, debug=False)
    x_d = nc.dram_tensor("x", [rows_per_core, NF], mybir.dt.float32,
                         kind="ExternalInput")
    bd_d = nc.dram_tensor("bd", [128, 128], mybir.dt.bfloat16,
                          kind="ExternalInput")
    hb_d = nc.dram_tensor("hb", [128, 128], mybir.dt.bfloat16,
                          kind="ExternalInput")
    y_d = nc.dram_tensor("y", [rows_per_core, NF], mybir.dt.float32,
                         kind="ExternalOutput")

    with tile.TileContext(nc) as tc:
        with (
            tc.tile_pool(name="consts", bufs=1) as cpool,
            tc.tile_pool(name="vin", bufs=v_bufs) as vpool,
            tc.tile_pool(name="amid", bufs=a_bufs) as apool,
            tc.tile_pool(name="yout", bufs=y_bufs) as ypool,
            tc.tile_pool(name="ps1", bufs=ps_bufs, space="PSUM") as ps1pool,
            tc.tile_pool(name="ps2", bufs=ps_bufs, space="PSUM") as ps2pool,
        ):
            bd_sb = cpool.tile([128, 128], mybir.dt.bfloat16)
            nc.sync.dma_start(bd_sb[:], bd_d.ap())
            hb_sb = cpool.tile([128, 128], mybir.dt.bfloat16)
            nc.sync.dma_start(hb_sb[:], hb_d.ap())

            def body(_it=None):
                for t in range(n_mega):
                    r0 = t * mega_rows
                    if mode in ("dmacontig", "dmamix"):
                        v = vpool.tile([128, FM], mybir.dt.bfloat16,
                                       tag="v")
                        if mode == "dmamix":
                            xi = x_d.ap()[r0:r0 + mega_rows, :].rearrange(
                                "(r4 rl) (a mc) -> (rl a) r4 mc",
                                rl=4, a=32, r4=R4, mc=128)
                            nc.gpsimd.dma_start(
                                v[:].rearrange("p (r4 mc) -> p r4 mc",
                                               r4=R4, mc=128), xi)
                        else:
                            nc.gpsimd.dma_start(
                                v[:], x_d.ap()[r0:r0 + mega_rows, :])
                        y_sb = ypool.tile([128, FM], mybir.dt.float32,
                                          tag="y")
                        nc.vector.memset(y_sb[:], 0.0)
                        nc.sync.dma_start(
                            y_d.ap()[r0:r0 + mega_rows, :], y_sb[:])
                        continue
                    xi = x_d.ap()[r0:r0 + mega_rows, :].rearrange(
                        "(r4 rl) (a mc) -> (rl a) r4 mc",
                        rl=4, a=32, r4=R4, mc=128)
                    v = vpool.tile([128, FM], mybir.dt.bfloat16, tag="v")
                    nc.gpsimd.dma_start(
                        v[:].rearrange("p (r4 mc) -> p r4 mc",
                                       r4=R4, mc=128), xi)

                    y_sb = ypool.tile([128, FM], mybir.dt.float32, tag="y")
                    if mode == "dma":
                        nc.vector.memset(y_sb[:], 0.0)
                    if contig_out and mode == "full":
                        # A layout: [mc, (a', r4, rl')] so stage-2 lhsT is a
                        # contiguous 128-slice per a' (rows for one a').
                        # drain1 does the (j, a', rl') -> (a', j, rl')
                        # reorder; matmul stationary APs allow only 1 free
                        # dim, engine copies allow many.
                        a_sb = apool.tile([128, FM], mybir.dt.bfloat16,
                                          tag="a")
                        a_v4 = a_sb[:].rearrange(
                            "p (ap r4 rl) -> p ap r4 rl",
                            ap=32, r4=R4, rl=4)
                        for q in range(n_chunk):
                            ps1 = ps1pool.tile([128, FC], mybir.dt.float32,
                                               tag="ps1")
                            for j in range(r4_chunk):
                                g = q * r4_chunk + j
                                nc.tensor.matmul(
                                    ps1[:, j * 128:(j + 1) * 128],
                                    v[:, g * 128:(g + 1) * 128], bd_sb[:])
                            nc.scalar.copy(
                                a_v4[:, :, q * r4_chunk:(q + 1) * r4_chunk,
                                     :],
                                ps1[:].rearrange(
                                    "p (j ap rl) -> p ap j rl",
                                    j=r4_chunk, ap=32, rl=4))
                        # stage 2: output partitions are ROWS; y_sb is
                        # row-major and the DMA-out is fully contiguous
                        apc = 1024 // 128  # a' values per PSUM tile
                        for qq in range(32 // apc):
                            ps2 = ps2pool.tile([128, apc * 128],
                                               mybir.dt.float32, tag="ps2")
                            for k in range(apc):
                                ap_idx = qq * apc + k
                                nc.tensor.matmul(
                                    ps2[:, k * 128:(k + 1) * 128],
                                    a_sb[:, ap_idx * 128:
                                         (ap_idx + 1) * 128],
                                    hb_sb[:])
                            nc.vector.tensor_copy(
                                y_sb[:, qq * apc * 128:(qq + 1) * apc * 128],
                                ps2[:])
                    else:
                        for q in range(n_chunk if mode != "dma" else 0):
                            ps1 = ps1pool.tile([128, FC], mybir.dt.float32,
                                               tag="ps1")
                            for j in range(r4_chunk):
                                g = q * r4_chunk + j
                                nc.tensor.matmul(
                                    ps1[:, j * 128:(j + 1) * 128],
                                    v[:, g * 128:(g + 1) * 128], bd_sb[:])
                            if mode == "t1":
                                nc.scalar.copy(y_sb[:, q * FC:(q + 1) * FC],
                                               ps1[:])
                                continue
                            a_sb = apool.tile([128, FC], mybir.dt.bfloat16,
                                              tag="a")
                            nc.scalar.copy(a_sb[:], ps1[:])

                            ps2 = ps2pool.tile([128, FC], mybir.dt.float32,
                                               tag="ps2")
                            for j in range(r4_chunk):
                                nc.tensor.matmul(
                                    ps2[:, j * 128:(j + 1) * 128],
                                    a_sb[:, j * 128:(j + 1) * 128],
                                    hb_sb[:])
                            nc.vector.tensor_copy(
                                y_sb[:, q * FC:(q + 1) * FC], ps2[:])

                    if contig_out and mode in ("full", "dma", "dmamix"):
                        nc.sync.dma_start(y_d.ap()[r0:r0 + mega_rows, :],
                                          y_sb[:])
                    else:
                        yo = y_d.ap()[r0:r0 + mega_rows, :].rearrange(
                            "(r4 rl) (a mc) -> (rl a) r4 mc",
                            rl=4, a=32, r4=R4, mc=128)
                        ysrc = y_sb[:].rearrange(
                            "p (r4 mc) -> p r4 mc", r4=R4, mc=128)
                        if out_dge == "sync":
                            nc.sync.dma_start(yo, ysrc)
                        else:
                            nc.gpsimd.dma_start(yo, ysrc)

            if reps == 1:
                body()
            else:
                with tc.For_i(0, reps, 1) as it:
                    body(it)

    nc.compile()
    return nc


_NC_CACHE = {}


def kernel(x):
    from concourse.bass_utils import run_bass_kernel_spmd

    x = np.asarray(x, dtype=np.float32)
    assert x.shape == (ROWS_TOTAL, NF)
    if "nc" not in _NC_CACHE:
        _NC_CACHE["nc"] = build_kernel()
    nc = _NC_CACHE["nc"]
    bd, hb = make_consts()
    shards = x.reshape(N_CORES, ROWS_PER_CORE, NF)
    in_maps = [
        {"x": np.ascontiguousarray(shards[i]), "bd": bd, "hb": hb}
        for i in range(N_CORES)
    ]
    res = run_bass_kernel_spmd(nc, in_maps, core_ids=list(range(N_CORES)))
    y = np.concatenate([res.results[i]["y"] for i in range(N_CORES)], axis=0)
    return np.asarray(y, dtype=np.float32)
